# revision 23
# baseline (speedup 1.0000x reference)
"""Trainium2 Bass kernel for nn_BasicConv2d (int8 conv + global requant + BN + requant + ReLU).

Self-contained: takes full inputs, shards batch dim over 8 NeuronCores,
runs one SPMD Bass program (conv as 9 shifted matmuls, tiny AllGathers for
the global max / BN-stat reductions), gathers full output.
"""
import numpy as np
import ml_dtypes

import jax  # noqa: F401  (axon PJRT backend provides the 8 NeuronCores)

try:
    jax.config.update("jax_compilation_cache_dir", "/tmp/jaxcache")
    jax.config.update("jax_persistent_cache_min_compile_time_secs", 0.0)
except Exception:
    pass

import concourse.bass as bass
import concourse.bass_isa as bass_isa
import concourse.tile as tile
from concourse import mybir, bacc
from concourse.bass_utils import run_bass_kernel_spmd

F32 = mybir.dt.float32
I32 = mybir.dt.int32
I8 = mybir.dt.int8
BF16 = mybir.dt.bfloat16
AF = mybir.ActivationFunctionType
OP = mybir.AluOpType
AX = mybir.AxisListType

N, CIN, H, W = 32, 128, 56, 56
COUT, KH, KW = 256, 3, 3
OH, OW = 54, 54
PX = OH * OW            # 2916
NCORES = 8
NIMG = N // NCORES      # 4 images per core
NRB = 6                 # row blocks per image (9 output rows each)
RBPX = PX // NRB        # 486 = 9 rows * 54 cols
HALFS = 2               # two 128-channel halves of COUT
COLS_H = NIMG * PX      # 11664 columns per half
COLS = HALFS * COLS_H   # 23328
EPS = 1e-5
RG = [list(range(NCORES))]
CC1 = 520               # [0:256)=chmax, [256:512)=chmin, [512]=local r1
# phase-3 column split per (half, image) chunk of 2916
P3_ACT, P3_DVE, P3_GPS = 1600, 816, 500

_cached = {}


def _col(h, i, rb=0):
    return (h * NIMG + i) * PX + rb * RBPX


def _bitexp_pow2(nc, pool, r_ap, name, p=128):
    """r [p,1] f32 (>0) -> (s [p,1] f32 = 2^(7-ceil(log2 r)),
    bwb [p,1] i32 = ceil(log2 r) + 127). Exact bit arithmetic."""
    ri = r_ap.bitcast(I32)
    eb = pool.tile([p, 1], I32, tag=f"{name}_eb")
    nc.vector.tensor_scalar(eb[:], ri, 23, 0xFF, OP.logical_shift_right, OP.bitwise_and)
    mant = pool.tile([p, 1], I32, tag=f"{name}_mant")
    nc.vector.tensor_scalar(mant[:], ri, 0x7FFFFF, None, OP.bitwise_and)
    nz = pool.tile([p, 1], I32, tag=f"{name}_nz")
    nc.vector.tensor_scalar(nz[:], mant[:], 0, None, OP.is_gt)
    bwb = pool.tile([p, 1], I32, tag=f"{name}_bwb")
    nc.vector.tensor_tensor(bwb[:], eb[:], nz[:], OP.add)
    t = pool.tile([p, 1], I32, tag=f"{name}_t")
    nc.vector.tensor_scalar(t[:], bwb[:], -1, 261, OP.mult, OP.add)  # 261 - bwb
    sb = pool.tile([p, 1], I32, tag=f"{name}_sb")
    nc.vector.tensor_scalar(sb[:], t[:], 23, None, OP.logical_shift_left)
    s = pool.tile([p, 1], F32, tag=f"{name}_s")
    nc.vector.tensor_copy(s[:], sb[:].bitcast(F32))
    return s, bwb


def _pow2_from_int(nc, pool, oi_ap, name, p=128):
    """2^k for k given as [p,1] int32 (normal range)."""
    b = pool.tile([p, 1], I32, tag=f"{name}_b")
    nc.vector.tensor_scalar(b[:], oi_ap, 127, None, OP.add)
    bs = pool.tile([p, 1], I32, tag=f"{name}_bs")
    nc.vector.tensor_scalar(bs[:], b[:], 23, None, OP.logical_shift_left)
    pt = pool.tile([p, 1], F32, tag=f"{name}_p")
    nc.vector.tensor_copy(pt[:], bs[:].bitcast(F32))
    return pt


def _build():
    nc = bacc.Bacc("TRN2", target_bir_lowering=False, debug=False, num_devices=NCORES)

    x_in = nc.dram_tensor("x", [NIMG, CIN, H * W], BF16, kind="ExternalInput")
    w_in = nc.dram_tensor("w", [KH * KW, CIN, COUT], BF16, kind="ExternalInput")
    scal_in = nc.dram_tensor("scal", [1, 1], F32, kind="ExternalInput")  # x_exp+w_exp
    gamma_in = nc.dram_tensor("gamma2", [HALFS, 128], F32, kind="ExternalInput")
    beta_in = nc.dram_tensor("beta2", [HALFS, 128], F32, kind="ExternalInput")
    out_val = nc.dram_tensor("out_val", [NIMG, COUT, PX], I8, kind="ExternalOutput")
    out_exp = nc.dram_tensor("out_exp", [1, 1], F32, kind="ExternalOutput")

    with tile.TileContext(nc) as tc:
        with (
            tc.tile_pool(name="big", bufs=1) as big,
            tc.tile_pool(name="stat", bufs=1) as stat,
            tc.tile_pool(name="dram", bufs=1, space="DRAM") as dram,
            tc.tile_pool(name="psum", bufs=2, space="PSUM") as psum_pool,
        ):
            # ---- PE warmup: dummy matmuls on never-written SBUF, overlaps input DMA
            dummy = big.tile([128, 128], BF16)
            nc.vector.memset(dummy[:], 1.0)
            wps = psum_pool.tile([128, 128], F32, tag="ps0")
            for _ in range(36):
                nc.tensor.matmul(wps[:], dummy[:], dummy[:], start=True, stop=True)

            # ---- load inputs to SBUF ----
            # x on the HWDGE (sync) queue; weights/params on SWDGE (gpsimd)
            # so the transfers run in parallel. Image 0 split at row 29 --
            # exactly what conv row-block group 0 needs to start.
            w_sb = big.tile([128, KH * KW, COUT], BF16)
            nc.gpsimd.dma_start(w_sb[:], w_in[:].rearrange("k p c -> p k c"))
            x_sb = big.tile([128, NIMG, H * W], BF16)
            XC = 29 * W
            nc.sync.dma_start(x_sb[:, 0, 0:XC], x_in[0, :, 0:XC])
            nc.sync.dma_start(x_sb[:, 0, XC:], x_in[0, :, XC:])
            for i in range(1, NIMG):
                nc.sync.dma_start(x_sb[:, i, :], x_in[i])
            gam_sb = stat.tile([128, HALFS], F32)
            nc.gpsimd.dma_start(gam_sb[:], gamma_in[:].rearrange("h p -> p h"))
            bet_sb = stat.tile([128, HALFS], F32)
            nc.gpsimd.dma_start(bet_sb[:], beta_in[:].rearrange("h p -> p h"))
            scal_sb = stat.tile([128, 1], F32)
            nc.gpsimd.dma_start(
                scal_sb[:],
                bass.AP(tensor=scal_in, offset=0, ap=[[0, 128], [1, 1]]),
            )

            acc_sb = big.tile([128, COLS], F32)
            q_sb = big.tile([128, COLS], I8)
            o_sb = big.tile([128, COLS], I8)

            mx_raw = stat.tile([128, HALFS, NIMG * NRB], F32)
            mn_raw = stat.tile([128, HALFS, NIMG * NRB], F32)

            # ---- phase 1: conv; G row-blocks share one weight load per k ----
            G = 3
            for i in range(NIMG):
                x_img = x_sb[:, i, :].rearrange("p (r c) -> p r c", c=W)
                for rbg in range(0, NRB, G):
                    for h in range(HALFS):
                        pss = []
                        for g in range(G):
                            ps = psum_pool.tile([128, RBPX], F32, tag=f"ps{g}")
                            pss.append(ps)
                        for k in range(KH * KW):
                            kh, kw = divmod(k, KW)
                            for g in range(G):
                                rb = rbg + g
                                rhs = x_img[:, rb * 9 + kh : rb * 9 + kh + 9,
                                            kw : kw + OW]
                                nc.tensor.matmul(
                                    pss[g][:],
                                    w_sb[:, k, h * 128 : (h + 1) * 128],
                                    rhs,
                                    start=(k == 0),
                                    stop=(k == KH * KW - 1),
                                )
                        for g in range(G):
                            rb = rbg + g
                            c0 = _col(h, i, rb)
                            nc.scalar.activation(acc_sb[:, c0 : c0 + RBPX],
                                                 pss[g][:], AF.Copy)
                            j = i * NRB + rb
                            nc.vector.tensor_reduce(
                                mx_raw[:, h, j : j + 1], pss[g][:], AX.X, OP.max
                            )
                            nc.vector.tensor_reduce(
                                mn_raw[:, h, j : j + 1], pss[g][:], AX.X, OP.min
                            )

            # per-core per-channel acc max/min, packed [kind(2), half(2)] per col
            chmm = stat.tile([128, 2, HALFS], F32)
            for h in range(HALFS):
                nc.vector.tensor_reduce(chmm[:, 0, h : h + 1], mx_raw[:, h, :],
                                        AX.X, OP.max)
                nc.vector.tensor_reduce(chmm[:, 1, h : h + 1], mn_raw[:, h, :],
                                        AX.X, OP.min)
            # local r1 on all partitions (gpsimd cross-partition absmax)
            rloc = stat.tile([128, 1], F32)
            nc.vector.tensor_reduce(rloc[:], chmm[:], AX.XY, OP.max,
                                    apply_absolute_value=True)
            r1c = stat.tile([128, 1], F32)
            nc.gpsimd.partition_all_reduce(r1c[:], rloc[:], 128,
                                           bass_isa.ReduceOp.max)

            # ---- collective 1: AllGather per-channel acc max/min + local r1 ----
            cc1_in = dram.tile([CC1], F32)
            nc.sync.dma_start(
                bass.AP(tensor=cc1_in.tensor, offset=cc1_in[:].offset,
                        ap=[[1, 128], [256, 2], [128, HALFS]]),
                chmm[:],
            )
            nc.sync.dma_start(cc1_in[512:513], r1c[0:1, 0:1])

            cc1_out = dram.tile([NCORES, CC1], F32)
            nc.gpsimd.collective_compute(
                "AllGather", OP.bypass, replica_groups=RG,
                ins=[cc1_in[:].opt()], outs=[cc1_out[:].opt()],
            )

            # r1 on all partitions: broadcast-read the 8 per-core r1 slots
            r1g = stat.tile([128, NCORES], F32)
            nc.sync.dma_start(
                r1g[:],
                bass.AP(tensor=cc1_out.tensor, offset=cc1_out[:].offset + 512,
                        ap=[[0, 128], [CC1, NCORES]]),
            )
            r1 = stat.tile([128, 1], F32)
            nc.vector.tensor_reduce(r1[:], r1g[:], AX.X, OP.max,
                                    apply_absolute_value=True)
            r1m = stat.tile([128, 1], F32)
            nc.vector.tensor_scalar(r1m[:], r1[:], 1.0, None, OP.max)
            # r1 is integer-valued: ceil(log2 r) = floor(log2(2r-1)) = expfield-127
            t2r = stat.tile([128, 1], F32)
            nc.vector.tensor_scalar(t2r[:], r1m[:], 2.0, -1.0, OP.mult, OP.add)
            bwb1 = stat.tile([128, 1], I32)
            nc.vector.tensor_scalar(bwb1[:], t2r[:].bitcast(I32), 23, 0xFF,
                                    OP.logical_shift_right, OP.bitwise_and)
            s1i = stat.tile([128, 1], I32)
            nc.vector.tensor_scalar(s1i[:], bwb1[:], -1, 261, OP.mult, OP.add)
            nc.vector.tensor_scalar(s1i[:], s1i[:], 23, None, OP.logical_shift_left)
            s1_b = stat.tile([128, 1], F32)
            nc.vector.tensor_copy(s1_b[:], s1i[:].bitcast(F32))

            # oe = (x_exp+w_exp) + bw1 - 7 ;  poe = 2^oe, poe2 = 2^(2*oe)
            sxw_i = stat.tile([128, 1], I32)
            nc.vector.tensor_copy(sxw_i[:], scal_sb[:])
            oe_i = stat.tile([128, 1], I32)
            nc.vector.tensor_scalar(oe_i[:], bwb1[:], 1, -134, OP.mult, OP.add)
            nc.vector.tensor_tensor(oe_i[:], oe_i[:], sxw_i[:], OP.add)
            oe2_i = stat.tile([128, 1], I32)
            nc.vector.tensor_scalar(oe2_i[:], oe_i[:], 2, None, OP.mult)
            poe_b = _pow2_from_int(nc, stat, oe_i[:], "poe")
            poe2_b = _pow2_from_int(nc, stat, oe2_i[:], "poe2")

            # ---- phase 2: q = int8(acc * s1) ; bn stats of q ----
            stats6 = stat.tile([128, HALFS, NIMG * NRB, 6], F32)
            for h in range(HALFS):
                for i in range(NIMG):
                    c0 = _col(h, i)
                    if h == 0 and i == 0:
                        # fine-grained first chunk: lets bn_stats start ~2us sooner
                        for rb in range(NRB):
                            cb = c0 + rb * RBPX
                            nc.scalar.activation(
                                q_sb[:, cb : cb + RBPX], acc_sb[:, cb : cb + RBPX],
                                AF.Copy, scale=s1_b[:, 0:1],
                            )
                    else:
                        nc.scalar.activation(
                            q_sb[:, c0 : c0 + PX], acc_sb[:, c0 : c0 + PX],
                            AF.Copy, scale=s1_b[:, 0:1],
                        )
                    for rb in range(NRB):
                        cb = c0 + rb * RBPX
                        nc.vector.bn_stats(
                            stats6[:, h, i * NRB + rb, :], q_sb[:, cb : cb + RBPX]
                        )
            # global per-channel acc extremes -> q extremes
            gmm = stat.tile([128, 2 * HALFS, NCORES], F32)
            for kh in range(2 * HALFS):
                nc.sync.dma_start(
                    gmm[:, kh, :],
                    bass.AP(tensor=cc1_out.tensor,
                            offset=cc1_out[:].offset + kh * 128,
                            ap=[[1, 128], [CC1, NCORES]]),
                )
            gchmax = stat.tile([128, HALFS], F32)
            nc.vector.tensor_reduce(gchmax[:], gmm[:, 0:HALFS, :], AX.X, OP.max)
            gchmin = stat.tile([128, HALFS], F32)
            nc.vector.tensor_reduce(gchmin[:], gmm[:, HALFS : 2 * HALFS, :],
                                    AX.X, OP.min)
            qmx8 = stat.tile([128, HALFS], I8)
            nc.scalar.activation(qmx8[:], gchmax[:], AF.Copy, scale=s1_b[:, 0:1])
            qmn8 = stat.tile([128, HALFS], I8)
            nc.scalar.activation(qmn8[:], gchmin[:], AF.Copy, scale=s1_b[:, 0:1])
            qmaxf = stat.tile([128, HALFS], F32)
            nc.vector.tensor_copy(qmaxf[:], qmx8[:])
            qminf = stat.tile([128, HALFS], F32)
            nc.vector.tensor_copy(qminf[:], qmn8[:])

            mv = stat.tile([128, HALFS, 2], F32)
            for h in range(HALFS):
                nc.vector.bn_aggr(mv[:, h, :], stats6[:, h, :, :])
            # pre-scale to xf units (mean *= 2^oe, var *= 2^2oe) off critical path
            for h in range(HALFS):
                nc.scalar.activation(mv[:, h, 0:1], mv[:, h, 0:1], AF.Copy,
                                     scale=poe_b[:, 0:1])
                nc.scalar.activation(mv[:, h, 1:2], mv[:, h, 1:2], AF.Copy,
                                     scale=poe2_b[:, 0:1])

            # ---- collective 2: AllGather per-channel (mean, var) ----
            cc2_in = dram.tile([2 * HALFS * 128], F32)
            for h in range(HALFS):
                nc.sync.dma_start(cc2_in[h * 128 : (h + 1) * 128], mv[:, h, 0:1])
                nc.sync.dma_start(
                    cc2_in[256 + h * 128 : 256 + (h + 1) * 128], mv[:, h, 1:2]
                )
            cc2_out = dram.tile([NCORES, 2 * HALFS * 128], F32)
            nc.gpsimd.collective_compute(
                "AllGather", OP.bypass, replica_groups=RG,
                ins=[cc2_in[:].opt()], outs=[cc2_out[:].opt()],
            )
            gmv = stat.tile([128, 2 * HALFS, NCORES], F32)
            for kh in range(2 * HALFS):
                nc.sync.dma_start(
                    gmv[:, kh, :],
                    bass.AP(tensor=cc2_out.tensor,
                            offset=cc2_out[:].offset + kh * 128,
                            ap=[[1, 128], [2 * HALFS * 128, NCORES]]),
                )
            gmean = gmv[:, 0:HALFS, :]
            gvar = gmv[:, HALFS : 2 * HALFS, :]

            # combine: mean_g = avg(mean_i); var_g = avg(var_i + mean_i^2) - mean_g^2
            mean_g = stat.tile([128, HALFS], F32)
            nc.vector.tensor_reduce(mean_g[:], gmean, AX.X, OP.add)
            nc.vector.tensor_scalar(mean_g[:], mean_g[:], 1.0 / NCORES, None, OP.mult)
            m2t = stat.tile([128, HALFS, NCORES], F32)
            nc.vector.tensor_tensor(m2t[:], gmean, gmean, OP.mult)
            nc.vector.tensor_tensor(m2t[:], m2t[:], gvar, OP.add)
            ex2 = stat.tile([128, HALFS], F32)
            nc.vector.tensor_reduce(ex2[:], m2t[:], AX.X, OP.add)
            nc.vector.tensor_scalar(ex2[:], ex2[:], 1.0 / NCORES, None, OP.mult)
            var_g = stat.tile([128, HALFS], F32)
            nc.vector.tensor_tensor(var_g[:], mean_g[:], mean_g[:], OP.mult)
            nc.vector.tensor_tensor(var_g[:], ex2[:], var_g[:], OP.subtract)

            # rs = rsqrt(var_xf + eps), Newton-refined (var already in xf units)
            veps = stat.tile([128, HALFS], F32)
            nc.vector.tensor_scalar(veps[:], var_g[:], EPS, None, OP.add)
            eps_t = stat.tile([128, 1], F32)
            nc.vector.memset(eps_t[:], EPS)
            s_sq = stat.tile([128, HALFS], F32)
            nc.scalar.activation(s_sq[:], var_g[:], AF.Sqrt, bias=eps_t[:, 0:1])
            for it in range(1):
                rcp = stat.tile([128, HALFS], F32, tag=f"rcp{it}")
                nc.vector.reciprocal(rcp[:], s_sq[:])
                tn = stat.tile([128, HALFS], F32, tag=f"tn{it}")
                nc.vector.tensor_tensor(tn[:], veps[:], rcp[:], OP.mult)
                nc.vector.tensor_tensor(tn[:], tn[:], s_sq[:], OP.add)
                nc.vector.tensor_scalar(s_sq[:], tn[:], 0.5, None, OP.mult)
            rs = stat.tile([128, HALFS], F32)
            nc.vector.reciprocal(rs[:], s_sq[:])

            # A0 = 2^oe * rs * gamma ; B0 = beta - mean_g*2^oe * rs*gamma
            rg_t = stat.tile([128, HALFS], F32)
            nc.vector.tensor_tensor(rg_t[:], rs[:], gam_sb[:], OP.mult)
            a0 = stat.tile([128, HALFS], F32)
            nc.scalar.activation(a0[:], rg_t[:], AF.Copy, scale=poe_b[:, 0:1])
            u = stat.tile([128, HALFS], F32)
            nc.vector.tensor_tensor(u[:], mean_g[:], rg_t[:], OP.mult)
            b0 = stat.tile([128, HALFS], F32)
            nc.vector.tensor_tensor(b0[:], bet_sb[:], u[:], OP.subtract)

            # r2 = max_c max(|A0*qmax+B0|, |A0*qmin+B0|)
            c1 = stat.tile([128, HALFS], F32)
            c2 = stat.tile([128, HALFS], F32)
            for h in range(HALFS):
                nc.scalar.activation(c1[:, h : h + 1], qmaxf[:, h : h + 1], AF.Abs,
                                     bias=b0[:, h : h + 1], scale=a0[:, h : h + 1])
                nc.scalar.activation(c2[:, h : h + 1], qminf[:, h : h + 1], AF.Abs,
                                     bias=b0[:, h : h + 1], scale=a0[:, h : h + 1])
            chr2 = stat.tile([128, HALFS], F32)
            nc.vector.tensor_tensor(chr2[:], c1[:], c2[:], OP.max)
            rr2 = stat.tile([128, 1], F32)
            nc.vector.tensor_tensor(rr2[:], chr2[:, 0:1], chr2[:, 1:2], OP.max)
            r2 = stat.tile([128, 1], F32)
            nc.gpsimd.partition_all_reduce(r2[:], rr2[:], 128,
                                           bass_isa.ReduceOp.max)
            r2m = stat.tile([128, 1], F32)
            nc.vector.tensor_scalar(r2m[:], r2[:], 1e-30, None, OP.max)
            s2_b, bwb2 = _bitexp_pow2(nc, stat, r2m[:], "s2")

            # exp2 = bw2 - 7
            e2i = stat.tile([128, 1], I32)
            nc.vector.tensor_scalar(e2i[:], bwb2[:], 1, -134, OP.mult, OP.add)
            e2f = stat.tile([128, 1], F32)
            nc.vector.tensor_copy(e2f[:], e2i[:])
            nc.sync.dma_start(out_exp[:], e2f[0:1, 0:1])

            # A' = A0*s2, B' = B0*s2
            ap_ = stat.tile([128, HALFS], F32)
            nc.scalar.activation(ap_[:], a0[:], AF.Copy, scale=s2_b[:, 0:1])
            bp_ = stat.tile([128, HALFS], F32)
            nc.scalar.activation(bp_[:], b0[:], AF.Copy, scale=s2_b[:, 0:1])

            # ---- phase 3: out = int8(relu(A'*q + B')) on ACT, DMA out per chunk ----
            for h in range(HALFS):
                ah, bh = ap_[:, h : h + 1], bp_[:, h : h + 1]
                for i in range(NIMG):
                    c0 = _col(h, i)
                    nc.scalar.activation(
                        o_sb[:, c0 : c0 + PX], q_sb[:, c0 : c0 + PX],
                        AF.Relu, bias=bh, scale=ah,
                    )
                    nc.sync.dma_start(
                        out_val[i, h * 128 : (h + 1) * 128, :],
                        o_sb[:, c0 : c0 + PX],
                    )

    nc.finalize()
    _dedupe_ldweights(nc)
    _thin_sem_incs(nc)
    return nc


def _thin_sem_incs(nc):
    """Tile emits a sem increment on every engine op, but only increments whose
    cumulative value some wait targets are observable. Keep exactly those and
    remap wait thresholds to ranks; drop the rest (saves ~26-47ns NX time per
    dropped inc, mostly on the 432 conv matmuls)."""
    from collections import defaultdict

    seq = []
    for func in nc.m.functions:
        for bb in func.blocks:
            seq.extend(bb.instructions)

    upd = defaultdict(list)
    wts = defaultdict(list)
    bad = set()
    for ins in seq:
        si = ins.sync_info
        if not si:
            continue
        for u in si.on_update:
            if getattr(u, "sync_type", None) != "semaphore":
                bad.add(getattr(u, "id", -1))
                continue
            upd[u.id].append((ins, u))
            if (u.update_mode != "sem-inc" or u.update_value != 1
                    or getattr(u, "update_reg", None) is not None):
                bad.add(u.id)
        for w in si.on_wait:
            if getattr(w, "sync_type", None) != "semaphore":
                bad.add(getattr(w, "id", -1))
                continue
            wts[w.id].append((ins, w))
            if (w.wait_mode != "sem-ge-imm"
                    or getattr(w, "wait_reg", None) is not None
                    or w.wait_value is None):
                bad.add(w.id)

    removed = 0
    for sid, ulist in upd.items():
        if sid in bad:
            continue
        wlist = wts.get(sid, [])
        n = len(ulist)
        targets = sorted({w.wait_value for (_, w) in wlist if w.wait_value > 0})
        if targets and targets[-1] > n:
            continue  # inconsistent; leave untouched
        keep = set(targets)
        if len(keep) >= n:
            continue
        rank = {}
        r = 0
        for pos in range(1, n + 1):
            if pos in keep:
                r += 1
            rank[pos] = r
        for pos, (ins, u) in enumerate(ulist, start=1):
            if pos not in keep:
                si = ins.sync_info
                si.on_update = [x for x in si.on_update if x is not u]
                removed += 1
        for (ins, w) in wlist:
            if w.wait_value > 0:
                new_v = rank[w.wait_value]
                assert 1 <= new_v <= len(keep)
                try:
                    w.wait_value = new_v
                except Exception:
                    si = ins.sync_info
                    nw = type(w)(sync_type=w.sync_type, id=w.id,
                                 ant_name=w.ant_name, wait_mode=w.wait_mode,
                                 wait_value=new_v,
                                 wait_reg=getattr(w, "wait_reg", None))
                    si.on_wait = [nw if x is w else x for x in si.on_wait]
    return removed


def _dedupe_ldweights(nc):
    """Drop InstLdweights that reload the exact weights already resident in
    the PE array (Bacc emits one per matmul; G row-blocks share weights).
    Waits on a dropped load migrate to the next PE matmul."""
    total = 0
    for func in nc.m.functions:
        for bb in func.blocks:
            keep = []
            last_key = None
            pending_waits = []
            dropped = 0
            for ins in bb.instructions:
                if isinstance(ins, mybir.InstLdweights):
                    a = ins.ins[0]
                    key = (getattr(a, "memref", None), getattr(a, "offset", None),
                           str(getattr(a, "ap", None)), str(getattr(a, "dtype", None)))
                    si = ins.sync_info
                    ups = si.on_update if si else []
                    if key == last_key and key[0] is not None and not ups:
                        if si and si.on_wait:
                            pending_waits.extend(si.on_wait)
                        dropped += 1
                        continue  # drop this instruction
                    last_key = key
                elif isinstance(ins, mybir.InstMatmult):
                    if pending_waits:
                        si = ins.sync_info
                        if si is None:
                            ins.sync_info = mybir.SyncInfo(
                                on_wait=list(pending_waits), on_update=[])
                        else:
                            si.on_wait = list(si.on_wait) + pending_waits
                        pending_waits = []
                elif getattr(ins, "engine", None) == mybir.EngineType.PE:
                    last_key = None  # unknown PE op: don't reuse across it
                keep.append(ins)
            if dropped:
                assert not pending_waits, "dangling waits from dropped ldweights"
                del bb.instructions[:]
                for i in keep:
                    bb.instructions.append(i)
                total += dropped
    return total


def _get_nc():
    if "nc" not in _cached:
        _cached["nc"] = _build()
    return _cached["nc"]


def kernel(x_val, x_exp, w_val, w_exp, gamma, beta, _trace=False):
    nc = _get_nc()

    bf16 = ml_dtypes.bfloat16
    x = np.asarray(x_val).reshape(N, CIN, H * W).astype(bf16)
    # weights: [COUT, CIN, KH, KW] -> [KH*KW, CIN, COUT]
    w = np.ascontiguousarray(
        np.asarray(w_val).astype(np.float32).transpose(2, 3, 1, 0).reshape(KH * KW, CIN, COUT)
    ).astype(bf16)
    sxw = np.array([[np.float32(x_exp) + np.float32(w_exp)]], dtype=np.float32)
    g2 = np.ascontiguousarray(np.asarray(gamma, np.float32).reshape(HALFS, 128))
    b2 = np.ascontiguousarray(np.asarray(beta, np.float32).reshape(HALFS, 128))

    in_maps = []
    for c in range(NCORES):
        in_maps.append({
            "x": np.ascontiguousarray(x[c * NIMG : (c + 1) * NIMG]),
            "w": w,
            "scal": sxw,
            "gamma2": g2,
            "beta2": b2,
        })

    res = run_bass_kernel_spmd(nc, in_maps, list(range(NCORES)), trace=_trace)
    out = np.concatenate([res.results[c]["out_val"] for c in range(NCORES)], axis=0)
    out = out.reshape(N, COUT, OH, OW)
    exp2 = np.float32(res.results[0]["out_exp"][0, 0])
    if _trace:
        kernel.last_results = res
    return out, exp2


# revision 24
# speedup vs baseline: 1.0392x; 1.0392x over previous
"""Trainium2 Bass kernel for nn_BasicConv2d (int8 conv + global requant + BN + requant + ReLU).

Self-contained: takes full inputs, shards batch dim over 8 NeuronCores,
runs one SPMD Bass program (conv as 9 shifted matmuls, tiny AllGathers for
the global max / BN-stat reductions), gathers full output.
"""
import numpy as np
import ml_dtypes

import jax  # noqa: F401  (axon PJRT backend provides the 8 NeuronCores)

try:
    jax.config.update("jax_compilation_cache_dir", "/tmp/jaxcache")
    jax.config.update("jax_persistent_cache_min_compile_time_secs", 0.0)
except Exception:
    pass

import concourse.bass as bass
import concourse.bass_isa as bass_isa
import concourse.tile as tile
from concourse import mybir, bacc
from concourse.bass_utils import run_bass_kernel_spmd

F32 = mybir.dt.float32
I32 = mybir.dt.int32
I8 = mybir.dt.int8
BF16 = mybir.dt.bfloat16
AF = mybir.ActivationFunctionType
OP = mybir.AluOpType
AX = mybir.AxisListType

N, CIN, H, W = 32, 128, 56, 56
COUT, KH, KW = 256, 3, 3
OH, OW = 54, 54
PX = OH * OW            # 2916
NCORES = 8
NIMG = N // NCORES      # 4 images per core
NRB = 6                 # row blocks per image (9 output rows each)
RBPX = PX // NRB        # 486 = 9 rows * 54 cols
HALFS = 2               # two 128-channel halves of COUT
COLS_H = NIMG * PX      # 11664 columns per half
COLS = HALFS * COLS_H   # 23328
EPS = 1e-5
RG = [list(range(NCORES))]
CC1 = 520               # [0:256)=chmax, [256:512)=chmin, [512]=local r1
# phase-3 column split per (half, image) chunk of 2916
P3_ACT, P3_DVE, P3_GPS = 1600, 816, 500

_cached = {}


def _col(h, i, rb=0):
    return (h * NIMG + i) * PX + rb * RBPX


def _bitexp_pow2(nc, pool, r_ap, name, p=128):
    """r [p,1] f32 (>0) -> (s [p,1] f32 = 2^(7-ceil(log2 r)),
    bwb [p,1] i32 = ceil(log2 r) + 127). Exact bit arithmetic."""
    ri = r_ap.bitcast(I32)
    eb = pool.tile([p, 1], I32, tag=f"{name}_eb")
    nc.vector.tensor_scalar(eb[:], ri, 23, 0xFF, OP.logical_shift_right, OP.bitwise_and)
    mant = pool.tile([p, 1], I32, tag=f"{name}_mant")
    nc.vector.tensor_scalar(mant[:], ri, 0x7FFFFF, None, OP.bitwise_and)
    nz = pool.tile([p, 1], I32, tag=f"{name}_nz")
    nc.vector.tensor_scalar(nz[:], mant[:], 0, None, OP.is_gt)
    bwb = pool.tile([p, 1], I32, tag=f"{name}_bwb")
    nc.vector.tensor_tensor(bwb[:], eb[:], nz[:], OP.add)
    t = pool.tile([p, 1], I32, tag=f"{name}_t")
    nc.vector.tensor_scalar(t[:], bwb[:], -1, 261, OP.mult, OP.add)  # 261 - bwb
    sb = pool.tile([p, 1], I32, tag=f"{name}_sb")
    nc.vector.tensor_scalar(sb[:], t[:], 23, None, OP.logical_shift_left)
    s = pool.tile([p, 1], F32, tag=f"{name}_s")
    nc.vector.tensor_copy(s[:], sb[:].bitcast(F32))
    return s, bwb


def _pow2_from_int(nc, pool, oi_ap, name, p=128):
    """2^k for k given as [p,1] int32 (normal range)."""
    b = pool.tile([p, 1], I32, tag=f"{name}_b")
    nc.vector.tensor_scalar(b[:], oi_ap, 127, None, OP.add)
    bs = pool.tile([p, 1], I32, tag=f"{name}_bs")
    nc.vector.tensor_scalar(bs[:], b[:], 23, None, OP.logical_shift_left)
    pt = pool.tile([p, 1], F32, tag=f"{name}_p")
    nc.vector.tensor_copy(pt[:], bs[:].bitcast(F32))
    return pt


def _build():
    nc = bacc.Bacc("TRN2", target_bir_lowering=False, debug=False, num_devices=NCORES)

    x_in = nc.dram_tensor("x", [NIMG, CIN, H * W], BF16, kind="ExternalInput")
    w_in = nc.dram_tensor("w", [KH * KW, CIN, COUT], BF16, kind="ExternalInput")
    scal_in = nc.dram_tensor("scal", [1, 1], F32, kind="ExternalInput")  # x_exp+w_exp
    gamma_in = nc.dram_tensor("gamma2", [HALFS, 128], F32, kind="ExternalInput")
    beta_in = nc.dram_tensor("beta2", [HALFS, 128], F32, kind="ExternalInput")
    out_val = nc.dram_tensor("out_val", [NIMG, COUT, PX], I8, kind="ExternalOutput")
    out_exp = nc.dram_tensor("out_exp", [1, 1], F32, kind="ExternalOutput")

    with tile.TileContext(nc) as tc:
        with (
            tc.tile_pool(name="big", bufs=1) as big,
            tc.tile_pool(name="stat", bufs=1) as stat,
            tc.tile_pool(name="dram", bufs=1, space="DRAM") as dram,
            tc.tile_pool(name="psum", bufs=2, space="PSUM") as psum_pool,
        ):
            # ---- PE warmup: dummy matmuls on never-written SBUF, overlaps input DMA
            dummy = big.tile([128, 128], BF16)
            nc.vector.memset(dummy[:], 1.0)
            wps = psum_pool.tile([128, 128], F32, tag="ps0")
            for _ in range(36):
                nc.tensor.matmul(wps[:], dummy[:], dummy[:], start=True, stop=True)

            # ---- load inputs to SBUF ----
            # x on the HWDGE (sync) queue; weights/params on SWDGE (gpsimd)
            # so the transfers run in parallel. Image 0 split at row 29 --
            # exactly what conv row-block group 0 needs to start.
            w_sb = big.tile([128, KH * KW, COUT], BF16)
            nc.gpsimd.dma_start(w_sb[:], w_in[:].rearrange("k p c -> p k c"))
            x_sb = big.tile([128, NIMG, H * W], BF16)
            XC = 29 * W
            nc.sync.dma_start(x_sb[:, 0, 0:XC], x_in[0, :, 0:XC])
            nc.sync.dma_start(x_sb[:, 0, XC:], x_in[0, :, XC:])
            for i in range(1, NIMG):
                nc.sync.dma_start(x_sb[:, i, :], x_in[i])
            gam_sb = stat.tile([128, HALFS], F32)
            nc.gpsimd.dma_start(gam_sb[:], gamma_in[:].rearrange("h p -> p h"))
            bet_sb = stat.tile([128, HALFS], F32)
            nc.gpsimd.dma_start(bet_sb[:], beta_in[:].rearrange("h p -> p h"))
            scal_sb = stat.tile([128, 1], F32)
            nc.gpsimd.dma_start(
                scal_sb[:],
                bass.AP(tensor=scal_in, offset=0, ap=[[0, 128], [1, 1]]),
            )

            acc_sb = big.tile([128, COLS], F32)
            q_sb = big.tile([128, COLS], I8)
            o_sb = big.tile([128, COLS], I8)

            mx_raw = stat.tile([128, HALFS, NIMG * NRB], F32)
            mn_raw = stat.tile([128, HALFS, NIMG * NRB], F32)

            # ---- phase 1: conv; G row-blocks share one weight load per k ----
            G = 3
            for i in range(NIMG):
                x_img = x_sb[:, i, :].rearrange("p (r c) -> p r c", c=W)
                for rbg in range(0, NRB, G):
                    for h in range(HALFS):
                        pss = []
                        for g in range(G):
                            ps = psum_pool.tile([128, RBPX], F32, tag=f"ps{g}")
                            pss.append(ps)
                        for k in range(KH * KW):
                            kh, kw = divmod(k, KW)
                            for g in range(G):
                                rb = rbg + g
                                rhs = x_img[:, rb * 9 + kh : rb * 9 + kh + 9,
                                            kw : kw + OW]
                                nc.tensor.matmul(
                                    pss[g][:],
                                    w_sb[:, k, h * 128 : (h + 1) * 128],
                                    rhs,
                                    start=(k == 0),
                                    stop=(k == KH * KW - 1),
                                )
                        for g in range(G):
                            rb = rbg + g
                            c0 = _col(h, i, rb)
                            nc.scalar.activation(acc_sb[:, c0 : c0 + RBPX],
                                                 pss[g][:], AF.Copy)
                            j = i * NRB + rb
                            nc.vector.tensor_reduce(
                                mx_raw[:, h, j : j + 1], pss[g][:], AX.X, OP.max
                            )
                            nc.vector.tensor_reduce(
                                mn_raw[:, h, j : j + 1], pss[g][:], AX.X, OP.min
                            )

            # per-core per-channel acc max/min, packed [kind(2), half(2)] per col
            chmm = stat.tile([128, 2, HALFS], F32)
            for h in range(HALFS):
                nc.vector.tensor_reduce(chmm[:, 0, h : h + 1], mx_raw[:, h, :],
                                        AX.X, OP.max)
                nc.vector.tensor_reduce(chmm[:, 1, h : h + 1], mn_raw[:, h, :],
                                        AX.X, OP.min)
            # local r1 on all partitions (gpsimd cross-partition absmax)
            rloc = stat.tile([128, 1], F32)
            nc.vector.tensor_reduce(rloc[:], chmm[:], AX.XY, OP.max,
                                    apply_absolute_value=True)
            r1c = stat.tile([128, 1], F32)
            nc.gpsimd.partition_all_reduce(r1c[:], rloc[:], 128,
                                           bass_isa.ReduceOp.max)

            # ---- collective 1: AllGather per-channel acc max/min + local r1 ----
            cc1_in = dram.tile([CC1], F32)
            nc.sync.dma_start(
                bass.AP(tensor=cc1_in.tensor, offset=cc1_in[:].offset,
                        ap=[[1, 128], [256, 2], [128, HALFS]]),
                chmm[:],
            )
            nc.sync.dma_start(cc1_in[512:513], r1c[0:1, 0:1])

            cc1_out = dram.tile([NCORES, CC1], F32)
            nc.gpsimd.collective_compute(
                "AllGather", OP.bypass, replica_groups=RG,
                ins=[cc1_in[:].opt()], outs=[cc1_out[:].opt()],
            )

            # r1 on all partitions: broadcast-read the 8 per-core r1 slots
            r1g = stat.tile([128, NCORES], F32)
            nc.sync.dma_start(
                r1g[:],
                bass.AP(tensor=cc1_out.tensor, offset=cc1_out[:].offset + 512,
                        ap=[[0, 128], [CC1, NCORES]]),
            )
            r1 = stat.tile([128, 1], F32)
            nc.vector.tensor_reduce(r1[:], r1g[:], AX.X, OP.max,
                                    apply_absolute_value=True)
            r1m = stat.tile([128, 1], F32)
            nc.vector.tensor_scalar(r1m[:], r1[:], 1.0, None, OP.max)
            # r1 is integer-valued: ceil(log2 r) = floor(log2(2r-1)) = expfield-127
            t2r = stat.tile([128, 1], F32)
            nc.vector.tensor_scalar(t2r[:], r1m[:], 2.0, -1.0, OP.mult, OP.add)
            bwb1 = stat.tile([128, 1], I32)
            nc.vector.tensor_scalar(bwb1[:], t2r[:].bitcast(I32), 23, 0xFF,
                                    OP.logical_shift_right, OP.bitwise_and)
            s1i = stat.tile([128, 1], I32)
            nc.vector.tensor_scalar(s1i[:], bwb1[:], -1, 261, OP.mult, OP.add)
            nc.vector.tensor_scalar(s1i[:], s1i[:], 23, None, OP.logical_shift_left)
            s1_b = stat.tile([128, 1], F32)
            nc.vector.tensor_copy(s1_b[:], s1i[:].bitcast(F32))

            # oe = (x_exp+w_exp) + bw1 - 7 ;  poe = 2^oe, poe2 = 2^(2*oe)
            sxw_i = stat.tile([128, 1], I32)
            nc.vector.tensor_copy(sxw_i[:], scal_sb[:])
            oe_i = stat.tile([128, 1], I32)
            nc.vector.tensor_scalar(oe_i[:], bwb1[:], 1, -134, OP.mult, OP.add)
            nc.vector.tensor_tensor(oe_i[:], oe_i[:], sxw_i[:], OP.add)
            oe2_i = stat.tile([128, 1], I32)
            nc.vector.tensor_scalar(oe2_i[:], oe_i[:], 2, None, OP.mult)
            poe_b = _pow2_from_int(nc, stat, oe_i[:], "poe")
            poe2_b = _pow2_from_int(nc, stat, oe2_i[:], "poe2")

            # ---- phase 2: q = int8(acc * s1) ; bn stats of q ----
            stats6 = stat.tile([128, HALFS, NIMG * NRB, 6], F32)
            for h in range(HALFS):
                for i in range(NIMG):
                    c0 = _col(h, i)
                    if h == 0 and i == 0:
                        # fine-grained first chunk: lets bn_stats start ~2us sooner
                        for rb in range(NRB):
                            cb = c0 + rb * RBPX
                            nc.scalar.activation(
                                q_sb[:, cb : cb + RBPX], acc_sb[:, cb : cb + RBPX],
                                AF.Copy, scale=s1_b[:, 0:1],
                            )
                    else:
                        nc.scalar.activation(
                            q_sb[:, c0 : c0 + PX], acc_sb[:, c0 : c0 + PX],
                            AF.Copy, scale=s1_b[:, 0:1],
                        )
                    if h == HALFS - 1 and i == NIMG - 1:
                        # last chunk's stats on ACT via accum_out (sum, sumsq);
                        # rebalances DVE-bound phase 2. Dumps go to dead acc cols.
                        sm_acc = stat.tile([128, 1], F32)
                        nc.scalar.activation(
                            acc_sb[:, c0 : c0 + PX],
                            q_sb[:, c0 : c0 + PX], AF.Copy, accum_out=sm_acc[:])
                        sq_acc = stat.tile([128, 1], F32)
                        nc.scalar.activation(
                            acc_sb[:, c0 : c0 + PX],
                            q_sb[:, c0 : c0 + PX], AF.Square, accum_out=sq_acc[:])
                    else:
                        for rb in range(NRB):
                            cb = c0 + rb * RBPX
                            nc.vector.bn_stats(
                                stats6[:, h, i * NRB + rb, :],
                                q_sb[:, cb : cb + RBPX]
                            )
            # global per-channel acc extremes -> q extremes
            gmm = stat.tile([128, 2 * HALFS, NCORES], F32)
            for kh in range(2 * HALFS):
                nc.sync.dma_start(
                    gmm[:, kh, :],
                    bass.AP(tensor=cc1_out.tensor,
                            offset=cc1_out[:].offset + kh * 128,
                            ap=[[1, 128], [CC1, NCORES]]),
                )
            gchmax = stat.tile([128, HALFS], F32)
            nc.vector.tensor_reduce(gchmax[:], gmm[:, 0:HALFS, :], AX.X, OP.max)
            gchmin = stat.tile([128, HALFS], F32)
            nc.vector.tensor_reduce(gchmin[:], gmm[:, HALFS : 2 * HALFS, :],
                                    AX.X, OP.min)
            qmx8 = stat.tile([128, HALFS], I8)
            nc.scalar.activation(qmx8[:], gchmax[:], AF.Copy, scale=s1_b[:, 0:1])
            qmn8 = stat.tile([128, HALFS], I8)
            nc.scalar.activation(qmn8[:], gchmin[:], AF.Copy, scale=s1_b[:, 0:1])
            qmaxf = stat.tile([128, HALFS], F32)
            nc.vector.tensor_copy(qmaxf[:], qmx8[:])
            qminf = stat.tile([128, HALFS], F32)
            nc.vector.tensor_copy(qminf[:], qmn8[:])

            mv = stat.tile([128, HALFS, 2], F32)
            nc.vector.bn_aggr(mv[:, 0, :], stats6[:, 0, :, :])
            # half 1: aggregate the 18 DVE blocks, then merge the ACT-chunk sums
            NB1 = (NIMG - 1) * NRB      # 18 blocks
            C1 = NB1 * RBPX             # 8748
            CT = NIMG * PX              # 11664
            mvh1 = stat.tile([128, 2], F32)
            nc.vector.bn_aggr(mvh1[:], stats6[:, HALFS - 1, 0:NB1, :])
            t_a = stat.tile([128, 1], F32)
            nc.vector.tensor_tensor(t_a[:], mvh1[:, 0:1], mvh1[:, 0:1], OP.mult)
            nc.vector.tensor_tensor(t_a[:], mvh1[:, 1:2], t_a[:], OP.add)  # E18[x2]
            nc.vector.tensor_scalar(t_a[:], t_a[:], float(C1), None, OP.mult)
            nc.vector.tensor_tensor(t_a[:], t_a[:], sq_acc[:], OP.add)
            nc.vector.tensor_scalar(t_a[:], t_a[:], 1.0 / CT, None, OP.mult)  # Ex2
            t_e = stat.tile([128, 1], F32)
            nc.vector.tensor_scalar(t_e[:], mvh1[:, 0:1], float(C1), None, OP.mult)
            nc.vector.tensor_tensor(t_e[:], t_e[:], sm_acc[:], OP.add)
            nc.vector.tensor_scalar(mv[:, HALFS - 1, 0:1], t_e[:], 1.0 / CT,
                                    None, OP.mult)
            t_g = stat.tile([128, 1], F32)
            nc.vector.tensor_tensor(t_g[:], mv[:, HALFS - 1, 0:1],
                                    mv[:, HALFS - 1, 0:1], OP.mult)
            nc.vector.tensor_tensor(mv[:, HALFS - 1, 1:2], t_a[:], t_g[:],
                                    OP.subtract)
            # pre-scale to xf units (mean *= 2^oe, var *= 2^2oe) off critical path
            for h in range(HALFS):
                nc.scalar.activation(mv[:, h, 0:1], mv[:, h, 0:1], AF.Copy,
                                     scale=poe_b[:, 0:1])
                nc.scalar.activation(mv[:, h, 1:2], mv[:, h, 1:2], AF.Copy,
                                     scale=poe2_b[:, 0:1])

            # ---- collective 2: AllGather per-channel (mean, var) ----
            cc2_in = dram.tile([2 * HALFS * 128], F32)
            for h in range(HALFS):
                nc.sync.dma_start(cc2_in[h * 128 : (h + 1) * 128], mv[:, h, 0:1])
                nc.sync.dma_start(
                    cc2_in[256 + h * 128 : 256 + (h + 1) * 128], mv[:, h, 1:2]
                )
            cc2_out = dram.tile([NCORES, 2 * HALFS * 128], F32)
            nc.gpsimd.collective_compute(
                "AllGather", OP.bypass, replica_groups=RG,
                ins=[cc2_in[:].opt()], outs=[cc2_out[:].opt()],
            )
            gmv = stat.tile([128, 2 * HALFS, NCORES], F32)
            for kh in range(2 * HALFS):
                nc.sync.dma_start(
                    gmv[:, kh, :],
                    bass.AP(tensor=cc2_out.tensor,
                            offset=cc2_out[:].offset + kh * 128,
                            ap=[[1, 128], [2 * HALFS * 128, NCORES]]),
                )
            gmean = gmv[:, 0:HALFS, :]
            gvar = gmv[:, HALFS : 2 * HALFS, :]

            # combine: mean_g = avg(mean_i); var_g = avg(var_i + mean_i^2) - mean_g^2
            mean_g = stat.tile([128, HALFS], F32)
            nc.vector.tensor_reduce(mean_g[:], gmean, AX.X, OP.add)
            nc.vector.tensor_scalar(mean_g[:], mean_g[:], 1.0 / NCORES, None, OP.mult)
            m2t = stat.tile([128, HALFS, NCORES], F32)
            nc.vector.tensor_tensor(m2t[:], gmean, gmean, OP.mult)
            nc.vector.tensor_tensor(m2t[:], m2t[:], gvar, OP.add)
            ex2 = stat.tile([128, HALFS], F32)
            nc.vector.tensor_reduce(ex2[:], m2t[:], AX.X, OP.add)
            nc.vector.tensor_scalar(ex2[:], ex2[:], 1.0 / NCORES, None, OP.mult)
            var_g = stat.tile([128, HALFS], F32)
            nc.vector.tensor_tensor(var_g[:], mean_g[:], mean_g[:], OP.mult)
            nc.vector.tensor_tensor(var_g[:], ex2[:], var_g[:], OP.subtract)

            # rs = rsqrt(var_xf + eps), Newton-refined (var already in xf units)
            veps = stat.tile([128, HALFS], F32)
            nc.vector.tensor_scalar(veps[:], var_g[:], EPS, None, OP.add)
            eps_t = stat.tile([128, 1], F32)
            nc.vector.memset(eps_t[:], EPS)
            s_sq = stat.tile([128, HALFS], F32)
            nc.scalar.activation(s_sq[:], var_g[:], AF.Sqrt, bias=eps_t[:, 0:1])
            for it in range(1):
                rcp = stat.tile([128, HALFS], F32, tag=f"rcp{it}")
                nc.vector.reciprocal(rcp[:], s_sq[:])
                tn = stat.tile([128, HALFS], F32, tag=f"tn{it}")
                nc.vector.tensor_tensor(tn[:], veps[:], rcp[:], OP.mult)
                nc.vector.tensor_tensor(tn[:], tn[:], s_sq[:], OP.add)
                nc.vector.tensor_scalar(s_sq[:], tn[:], 0.5, None, OP.mult)
            rs = stat.tile([128, HALFS], F32)
            nc.vector.reciprocal(rs[:], s_sq[:])

            # A0 = 2^oe * rs * gamma ; B0 = beta - mean_g*2^oe * rs*gamma
            rg_t = stat.tile([128, HALFS], F32)
            nc.vector.tensor_tensor(rg_t[:], rs[:], gam_sb[:], OP.mult)
            a0 = stat.tile([128, HALFS], F32)
            nc.scalar.activation(a0[:], rg_t[:], AF.Copy, scale=poe_b[:, 0:1])
            u = stat.tile([128, HALFS], F32)
            nc.vector.tensor_tensor(u[:], mean_g[:], rg_t[:], OP.mult)
            b0 = stat.tile([128, HALFS], F32)
            nc.vector.tensor_tensor(b0[:], bet_sb[:], u[:], OP.subtract)

            # r2 = max_c max(|A0*qmax+B0|, |A0*qmin+B0|)
            c1 = stat.tile([128, HALFS], F32)
            c2 = stat.tile([128, HALFS], F32)
            for h in range(HALFS):
                nc.scalar.activation(c1[:, h : h + 1], qmaxf[:, h : h + 1], AF.Abs,
                                     bias=b0[:, h : h + 1], scale=a0[:, h : h + 1])
                nc.scalar.activation(c2[:, h : h + 1], qminf[:, h : h + 1], AF.Abs,
                                     bias=b0[:, h : h + 1], scale=a0[:, h : h + 1])
            chr2 = stat.tile([128, HALFS], F32)
            nc.vector.tensor_tensor(chr2[:], c1[:], c2[:], OP.max)
            rr2 = stat.tile([128, 1], F32)
            nc.vector.tensor_tensor(rr2[:], chr2[:, 0:1], chr2[:, 1:2], OP.max)
            r2 = stat.tile([128, 1], F32)
            nc.gpsimd.partition_all_reduce(r2[:], rr2[:], 128,
                                           bass_isa.ReduceOp.max)
            r2m = stat.tile([128, 1], F32)
            nc.vector.tensor_scalar(r2m[:], r2[:], 1e-30, None, OP.max)
            s2_b, bwb2 = _bitexp_pow2(nc, stat, r2m[:], "s2")

            # exp2 = bw2 - 7
            e2i = stat.tile([128, 1], I32)
            nc.vector.tensor_scalar(e2i[:], bwb2[:], 1, -134, OP.mult, OP.add)
            e2f = stat.tile([128, 1], F32)
            nc.vector.tensor_copy(e2f[:], e2i[:])
            nc.sync.dma_start(out_exp[:], e2f[0:1, 0:1])

            # A' = A0*s2, B' = B0*s2
            ap_ = stat.tile([128, HALFS], F32)
            nc.scalar.activation(ap_[:], a0[:], AF.Copy, scale=s2_b[:, 0:1])
            bp_ = stat.tile([128, HALFS], F32)
            nc.scalar.activation(bp_[:], b0[:], AF.Copy, scale=s2_b[:, 0:1])

            # ---- phase 3: out = int8(relu(A'*q + B')) on ACT, DMA out per chunk ----
            for h in range(HALFS):
                ah, bh = ap_[:, h : h + 1], bp_[:, h : h + 1]
                for i in range(NIMG):
                    c0 = _col(h, i)
                    nc.scalar.activation(
                        o_sb[:, c0 : c0 + PX], q_sb[:, c0 : c0 + PX],
                        AF.Relu, bias=bh, scale=ah,
                    )
                    nc.sync.dma_start(
                        out_val[i, h * 128 : (h + 1) * 128, :],
                        o_sb[:, c0 : c0 + PX],
                    )

    nc.finalize()
    _dedupe_ldweights(nc)
    _thin_sem_incs(nc)
    return nc


def _thin_sem_incs(nc):
    """Tile emits a sem increment on every engine op, but only increments whose
    cumulative value some wait targets are observable. Keep exactly those and
    remap wait thresholds to ranks; drop the rest (saves ~26-47ns NX time per
    dropped inc, mostly on the 432 conv matmuls)."""
    from collections import defaultdict

    seq = []
    for func in nc.m.functions:
        for bb in func.blocks:
            seq.extend(bb.instructions)

    upd = defaultdict(list)
    wts = defaultdict(list)
    bad = set()
    for ins in seq:
        si = ins.sync_info
        if not si:
            continue
        for u in si.on_update:
            if getattr(u, "sync_type", None) != "semaphore":
                bad.add(getattr(u, "id", -1))
                continue
            upd[u.id].append((ins, u))
            if (u.update_mode != "sem-inc" or u.update_value != 1
                    or getattr(u, "update_reg", None) is not None):
                bad.add(u.id)
        for w in si.on_wait:
            if getattr(w, "sync_type", None) != "semaphore":
                bad.add(getattr(w, "id", -1))
                continue
            wts[w.id].append((ins, w))
            if (w.wait_mode != "sem-ge-imm"
                    or getattr(w, "wait_reg", None) is not None
                    or w.wait_value is None):
                bad.add(w.id)

    removed = 0
    for sid, ulist in upd.items():
        if sid in bad:
            continue
        wlist = wts.get(sid, [])
        n = len(ulist)
        targets = sorted({w.wait_value for (_, w) in wlist if w.wait_value > 0})
        if targets and targets[-1] > n:
            continue  # inconsistent; leave untouched
        keep = set(targets)
        if len(keep) >= n:
            continue
        rank = {}
        r = 0
        for pos in range(1, n + 1):
            if pos in keep:
                r += 1
            rank[pos] = r
        for pos, (ins, u) in enumerate(ulist, start=1):
            if pos not in keep:
                si = ins.sync_info
                si.on_update = [x for x in si.on_update if x is not u]
                removed += 1
        for (ins, w) in wlist:
            if w.wait_value > 0:
                new_v = rank[w.wait_value]
                assert 1 <= new_v <= len(keep)
                try:
                    w.wait_value = new_v
                except Exception:
                    si = ins.sync_info
                    nw = type(w)(sync_type=w.sync_type, id=w.id,
                                 ant_name=w.ant_name, wait_mode=w.wait_mode,
                                 wait_value=new_v,
                                 wait_reg=getattr(w, "wait_reg", None))
                    si.on_wait = [nw if x is w else x for x in si.on_wait]
    return removed


def _dedupe_ldweights(nc):
    """Drop InstLdweights that reload the exact weights already resident in
    the PE array (Bacc emits one per matmul; G row-blocks share weights).
    Waits on a dropped load migrate to the next PE matmul."""
    total = 0
    for func in nc.m.functions:
        for bb in func.blocks:
            keep = []
            last_key = None
            pending_waits = []
            dropped = 0
            for ins in bb.instructions:
                if isinstance(ins, mybir.InstLdweights):
                    a = ins.ins[0]
                    key = (getattr(a, "memref", None), getattr(a, "offset", None),
                           str(getattr(a, "ap", None)), str(getattr(a, "dtype", None)))
                    si = ins.sync_info
                    ups = si.on_update if si else []
                    if key == last_key and key[0] is not None and not ups:
                        if si and si.on_wait:
                            pending_waits.extend(si.on_wait)
                        dropped += 1
                        continue  # drop this instruction
                    last_key = key
                elif isinstance(ins, mybir.InstMatmult):
                    if pending_waits:
                        si = ins.sync_info
                        if si is None:
                            ins.sync_info = mybir.SyncInfo(
                                on_wait=list(pending_waits), on_update=[])
                        else:
                            si.on_wait = list(si.on_wait) + pending_waits
                        pending_waits = []
                elif getattr(ins, "engine", None) == mybir.EngineType.PE:
                    last_key = None  # unknown PE op: don't reuse across it
                keep.append(ins)
            if dropped:
                assert not pending_waits, "dangling waits from dropped ldweights"
                del bb.instructions[:]
                for i in keep:
                    bb.instructions.append(i)
                total += dropped
    return total


def _get_nc():
    if "nc" not in _cached:
        _cached["nc"] = _build()
    return _cached["nc"]


def kernel(x_val, x_exp, w_val, w_exp, gamma, beta, _trace=False):
    nc = _get_nc()

    bf16 = ml_dtypes.bfloat16
    x = np.asarray(x_val).reshape(N, CIN, H * W).astype(bf16)
    # weights: [COUT, CIN, KH, KW] -> [KH*KW, CIN, COUT]
    w = np.ascontiguousarray(
        np.asarray(w_val).astype(np.float32).transpose(2, 3, 1, 0).reshape(KH * KW, CIN, COUT)
    ).astype(bf16)
    sxw = np.array([[np.float32(x_exp) + np.float32(w_exp)]], dtype=np.float32)
    g2 = np.ascontiguousarray(np.asarray(gamma, np.float32).reshape(HALFS, 128))
    b2 = np.ascontiguousarray(np.asarray(beta, np.float32).reshape(HALFS, 128))

    in_maps = []
    for c in range(NCORES):
        in_maps.append({
            "x": np.ascontiguousarray(x[c * NIMG : (c + 1) * NIMG]),
            "w": w,
            "scal": sxw,
            "gamma2": g2,
            "beta2": b2,
        })

    res = run_bass_kernel_spmd(nc, in_maps, list(range(NCORES)), trace=_trace)
    out = np.concatenate([res.results[c]["out_val"] for c in range(NCORES)], axis=0)
    out = out.reshape(N, COUT, OH, OW)
    exp2 = np.float32(res.results[0]["out_exp"][0, 0])
    if _trace:
        kernel.last_results = res
    return out, exp2
